# revision 18
# baseline (speedup 1.0000x reference)
"""Longformer attention TP-sharded Bass kernel for 8 NeuronCores (v2).

Sharding: tensor-parallel over heads. Core d owns heads 2d, 2d+1:
  - Wq/Wk/Wv rows [128d:128(d+1)]  (nn.Linear: q = x @ Wq.T)
  - Wo columns [128d:128(d+1)]
  Each core computes its heads' sparse (windowed+global) attention and a
  full-size out-proj partial; host sums the 8 partials (the "all-reduce").

v2 layout (all bf16 compute, fp32 PSUM accumulate):
  xT  [1024h, 4096s]   x transposed; DMA'd in 8 contiguous 1MB chunks
  qT/kT [128o, 4096s]  head dims on partitions (head A: 0-63, head B: 64-127)
  v   [128s, 32kb, 130] natural layout per key block: [vA | 1 | vB | 1]; the
                        ones columns make the PV matmul also emit the softmax
                        denominator.
  scoresT [k, q] per (qb, head) in one PSUM bank [128, 512]:
     [prev-block | next-block | diag-block | global-col strip(row 0, M=1 mm)]
  Global query row 0 (qb0) goes through M=1 strip matmuls so only the needed
  row is computed.  Masks are multiplicative 0/1 bf16 on exp(scores), 256
  cols wide, applied on the idle GpSimd engine.  Head A (PE rows 0-63) and
  head B (rows 64-127) score matmuls are emitted adjacently so the PE runs
  them concurrently (row-group tiling).
"""

import os
import numpy as np
import ml_dtypes

S = 4096
HIDDEN = 1024
N_CORES = 8
OC = 128          # out-proj contraction dims (head dims) per core = 2 heads x 64
NQB = S // 128    # 32 query/key blocks
BF16 = ml_dtypes.bfloat16

_CACHE = {}
LAST_RESULTS = None


def _masks_np():
    """Multiplicative masks [4, 128, 256] bf16, scoresT [k(part), q(free)]:
      0: interior qb: [keep f<=p (prev) | keep f>=p (next)]
      1: qb1:         [keep (f<=p)&(p>0) (kb0) | keep f>=p (kb2)]
      2: qb31:        [keep f<=p (kb30) | ones (diag)]
      3: qb0:         [keep (f>=p)|(f==0) (kb1) | ones (kb0)]
    """
    p = np.arange(128)[:, None]   # key index within block
    f = np.arange(128)[None, :]   # query index within block
    ones = np.ones((128, 128), bool)
    m_lo = (f <= p)
    m_lo_ng = m_lo & (p > 0)
    m_up = (f >= p)
    m_up0 = m_up | (f == 0)
    out = np.zeros((4, 128, 256), bool)
    out[0] = np.concatenate([m_lo, m_up], 1)
    out[1] = np.concatenate([m_lo_ng, m_up], 1)
    out[2] = np.concatenate([m_lo, ones], 1)
    out[3] = np.concatenate([m_up0, ones], 1)
    return out.astype(BF16)


def _band_for(qb):
    """[(key_block, col_offset)] band blocks for query block qb (qb >= 1)."""
    if qb == NQB - 1:
        return [(qb - 1, 0), (qb, 128)]
    return [(qb - 1, 0), (qb + 1, 128), (qb, 256)]


def _mask_cls(qb):
    if qb == 1:
        return 1
    if qb == NQB - 1:
        return 2
    return 0


def _build():
    import concourse.bass as bass
    import concourse.mybir as mybir
    import concourse.tile as tile
    from concourse import bacc

    f32 = mybir.dt.float32
    bf16 = mybir.dt.bfloat16
    Exp = mybir.ActivationFunctionType.Exp

    nc = bacc.Bacc("TRN2", target_bir_lowering=False, debug=False,
                   num_devices=N_CORES)

    # xt chunks: [sc][p][c][512] so each chunk is one contiguous 1MB transfer
    xt_d = nc.dram_tensor("xt", [8, 128, 8, 512], bf16, kind="ExternalInput").ap()
    # weights pre-arranged host-side: [p][c][o] contiguous
    wq_d = nc.dram_tensor("wqt", [128, 8, OC], bf16, kind="ExternalInput").ap()
    wk_d = nc.dram_tensor("wkt", [128, 8, OC], bf16, kind="ExternalInput").ap()
    wv_d = nc.dram_tensor("wvt", [128, 8, OC], bf16, kind="ExternalInput").ap()
    wo_d = nc.dram_tensor("wot", [OC, HIDDEN], bf16, kind="ExternalInput").ap()
    out_d = nc.dram_tensor("partial", [S, HIDDEN], bf16,
                           kind="ExternalOutput").ap()
    mask_d = nc.inline_tensor(
        np.ascontiguousarray(_masks_np().transpose(1, 0, 2)), name="masks").ap()
    id_d = nc.inline_tensor(np.eye(128, dtype=BF16), name="ident").ap()

    with tile.TileContext(nc) as tc:
        import contextlib
        with contextlib.ExitStack() as ctx:
            big = ctx.enter_context(tc.tile_pool(name="big", bufs=1))
            tmp = ctx.enter_context(tc.tile_pool(name="tmp", bufs=4))
            psS = ctx.enter_context(tc.tile_pool(name="psS", bufs=3, space="PSUM"))
            psP = ctx.enter_context(tc.tile_pool(name="psP", bufs=2, space="PSUM"))
            psT = ctx.enter_context(tc.tile_pool(name="psT", bufs=1, space="PSUM"))
            psO = ctx.enter_context(tc.tile_pool(name="psO", bufs=2, space="PSUM"))

            # ---- resident tensors ----
            xt_sb = big.tile([128, 8, 8, 512], bf16)  # x.T chunks [p, sc, c, s]
            qt_sb = big.tile([128, S], bf16)          # q.T (0.125 folded in Wq)
            kt_sb = big.tile([128, S], bf16)
            v_sb = big.tile([128, NQB, 130], bf16)    # [vA|1|vB|1] per key block
            outn_sb = big.tile([128, NQB, 128], bf16)  # attn out, natural [q, hd]
            outt_sb = big.tile([128, NQB, 128], bf16)  # transposed [hd, q]
            wq_sb = big.tile([128, 8, OC], bf16)
            wk_sb = big.tile([128, 8, OC], bf16)
            wv_sb = big.tile([128, 8, OC], bf16)
            wo_sb = big.tile([128, HIDDEN], bf16)
            mask_sb = big.tile([128, 4, 256], bf16)
            id_sb = big.tile([128, 128], bf16)

            scratch = big.tile([64, 64], bf16)    # never written: PE warmup fuel

            # ---- loads: wq + xt0 first so Q(sc0) starts asap ----
            nc.sync.dma_start(wq_sb, wq_d)
            nc.sync.dma_start(xt_sb[:, 0], xt_d[0])
            nc.sync.dma_start(wk_sb, wk_d)
            nc.sync.dma_start(wv_sb, wv_d)
            nc.sync.dma_start(xt_sb[:, 1], xt_d[1])
            nc.sync.dma_start(wo_sb, wo_d)
            nc.sync.dma_start(mask_sb, mask_d)
            nc.sync.dma_start(id_sb, id_d)
            nc.gpsimd.memset(v_sb[:, :, 64], 1.0)
            nc.gpsimd.memset(v_sb[:, :, 129], 1.0)
            nc.gpsimd.memset(scratch, 0.0)

            # keep the PE busy through the initial DMA wait so the HAM clock
            # gate is released (2.4 GHz) by the time real matmuls arrive
            psw = psT.tile([128, 128], f32, tag="tr", name="psw")
            for _ in range(90):
                nc.tensor.matmul(psw[0:64, 0:64], scratch, scratch,
                                 start=True, stop=True)

            q0_state = {}

            def q0_scores():
                # qb0: band [kb1 | kb0] + far strip (scores of q0 vs kb2..31)
                qsl = slice(0, 128)
                pss = [psS.tile([128, 512], f32, tag="s", name="pss")
                       for _ in range(2)]
                for kb, off in ((1, 0), (0, 128)):
                    for h in range(2):
                        bp = 64 * h
                        nc.tensor.matmul(
                            pss[h][:, off:off + 128],
                            kt_sb[bp:bp + 64, kb * 128:(kb + 1) * 128],
                            qt_sb[bp:bp + 64, qsl],
                            start=True, stop=True)
                for kb in range(2, NQB):
                    for h in range(2):
                        bp = 64 * h
                        nc.tensor.matmul(
                            pss[h][:, 254 + kb:255 + kb],
                            kt_sb[bp:bp + 64, kb * 128:(kb + 1) * 128],
                            qt_sb[bp:bp + 64, 0:1],
                            start=True, stop=True)
                probs = []
                for h in range(2):
                    # dedicated tag: these stay live across other qbs' rotations
                    pr = tmp.tile([128, 512], bf16, tag="probs0", bufs=2,
                                  name="pr0")
                    probs.append(pr)
                    nc.scalar.activation(pr[:, 0:286], pss[h][:, 0:286], Exp)
                    nc.gpsimd.tensor_mul(pr[:, 0:128], pr[:, 0:128],
                                         mask_sb[:, 3, 0:128])
                q0_state["probs"] = probs

            def attention(qb):
                qsl = slice(qb * 128, (qb + 1) * 128)
                pso = psP.tile([128, 512], f32, tag="pv", name="pso")
                probs = [None, None]
                if qb > 0:
                    band = _band_for(qb)
                    pss = [psS.tile([128, 512], f32, tag="s", name="pss")
                           for _ in range(2)]
                    # head-interleaved band matmuls: A rows 0-63, B rows 64-127
                    # run concurrently in the PE array (distinct row groups)
                    for kb, off in band:
                        for h in range(2):
                            bp = 64 * h
                            nc.tensor.matmul(
                                pss[h][:, off:off + 128],
                                kt_sb[bp:bp + 64, kb * 128:(kb + 1) * 128],
                                qt_sb[bp:bp + 64, qsl],
                                start=True, stop=True)
                    # global key-0 column strip: M=1, row 0 of cols 384:512
                    for h in range(2):
                        bp = 64 * h
                        nc.tensor.matmul(
                            pss[h][0:1, 384:512],
                            kt_sb[bp:bp + 64, 0:1],
                            qt_sb[bp:bp + 64, qsl],
                            start=True, stop=True)
                    for _ in range(6):
                        nc.tensor.matmul(pso[0:64, 256:320], scratch, scratch,
                                         start=True, stop=True,
                                         skip_group_check=True)
                    cls = _mask_cls(qb)
                    for h in range(2):
                        pr = tmp.tile([128, 512], bf16, tag="probs", name="pr")
                        probs[h] = pr
                        nc.scalar.activation(pr, pss[h], Exp)
                        nc.gpsimd.tensor_mul(pr[:, 0:256], pr[:, 0:256],
                                             mask_sb[:, cls, :])
                    # PV: probs stationary, v moving; ones cols give denoms.
                    # diag + key-0 strip first: they only need exp, not the
                    # mask, so the PE resumes sooner after the activation.
                    for h in range(2):
                        hsl = slice(65 * h, 65 * h + 65)
                        vsl = slice(65 * h, 65 * h + 65)
                        diag = band[-1]
                        nc.tensor.matmul(
                            pso[:, hsl], probs[h][:, diag[1]:diag[1] + 128],
                            v_sb[:, diag[0], vsl],
                            start=True, stop=False, skip_group_check=True)
                        # key-0 contribution: K=1 outer product
                        nc.tensor.matmul(
                            pso[:, hsl], probs[h][0:1, 384:512],
                            v_sb[0:1, 0, vsl],
                            start=False, stop=False, skip_group_check=True)
                        for j, (kb, off) in enumerate(band[:-1]):
                            nc.tensor.matmul(
                                pso[:, hsl], probs[h][:, off:off + 128],
                                v_sb[:, kb, vsl],
                                start=False, stop=(j == len(band) - 2),
                                skip_group_check=True)
                else:
                    probs = q0_state["probs"]
                    for h in range(2):
                        hsl = slice(65 * h, 65 * h + 65)
                        vsl = slice(65 * h, 65 * h + 65)
                        for j, (kb, off) in enumerate(((1, 0), (0, 128))):
                            nc.tensor.matmul(
                                pso[:, hsl], probs[h][:, off:off + 128],
                                v_sb[:, kb, vsl],
                                start=(j == 0), stop=False,
                                skip_group_check=True)
                        for kb in range(2, NQB):
                            nc.tensor.matmul(
                                pso[0:1, hsl],
                                probs[h][:, 254 + kb:255 + kb],
                                v_sb[:, kb, vsl],
                                start=False, stop=(kb == NQB - 1),
                                skip_group_check=True)

                # normalize + write outn
                recip = tmp.tile([128, 2], f32, tag="recip", name="recip")
                pso_h = pso[:, 0:130].rearrange("p (h c) -> p h c", h=2)
                nc.vector.reciprocal(recip, pso_h[:, :, 64])
                for h in range(2):
                    nc.vector.tensor_scalar_mul(
                        outn_sb[:, qb, 64 * h:64 * h + 64],
                        pso[:, 65 * h:65 * h + 64], recip[:, h:h + 1])

                # transpose -> out-proj -> stage -> DMA
                pstr = psT.tile([128, 128], bf16, tag="tr", name="pstr")
                nc.tensor.transpose(pstr, outn_sb[:, qb, :], id_sb)
                nc.vector.tensor_copy(outt_sb[:, qb, :], pstr)
                stage = tmp.tile([128, HIDDEN], bf16, tag="stage", name="stage")
                for oc in range(2):
                    psp = psO.tile([128, 512], f32, tag="o", name="psp")
                    nc.tensor.matmul(psp, outt_sb[:, qb, :],
                                     wo_sb[:, oc * 512:(oc + 1) * 512],
                                     start=True, stop=True)
                    if oc == 0:
                        nc.vector.tensor_copy(
                            stage[:, oc * 512:(oc + 1) * 512], psp)
                    else:
                        nc.scalar.copy(stage[:, oc * 512:(oc + 1) * 512], psp)
                nc.sync.dma_start(out_d[qb * 128:(qb + 1) * 128, :], stage)

            # ---- projections interleaved with attention ----
            done = 0
            for sc in range(8):
                if 2 <= sc + 1 <= 7:
                    nc.sync.dma_start(xt_sb[:, sc + 1], xt_d[sc + 1])
                ssl = slice(sc * 512, (sc + 1) * 512)

                psq = psS.tile([128, 512], f32, tag="s", name="psq")
                for hc in range(8):
                    nc.tensor.matmul(psq, wq_sb[:, hc, :], xt_sb[:, sc, hc],
                                     start=(hc == 0), stop=(hc == 7))
                nc.vector.tensor_copy(qt_sb[:, ssl], psq)

                psk = psS.tile([128, 512], f32, tag="s", name="psk")
                for hc in range(8):
                    nc.tensor.matmul(psk, wk_sb[:, hc, :], xt_sb[:, sc, hc],
                                     start=(hc == 0), stop=(hc == 7))
                nc.vector.tensor_copy(kt_sb[:, ssl], psk)

                psv = psS.tile([128, 512], f32, tag="s", name="psv")
                for b in range(4):
                    bsl = slice(b * 128, b * 128 + 128)
                    for hc in range(8):
                        nc.tensor.matmul(psv[:, b * 128:b * 128 + 128],
                                         xt_sb[:, sc, hc, bsl], wv_sb[:, hc, :],
                                         start=(hc == 0), stop=(hc == 7),
                                         skip_group_check=True)
                # one strided copy for all 4 blocks x 2 heads
                vdst = v_sb[:, sc * 4:sc * 4 + 4, :].rearrange(
                    "p b (h c) -> p b h c", h=2)
                vsrc = psv.rearrange("p (b h c) -> p b h c", b=4, h=2)
                nc.vector.tensor_copy(vdst[:, :, :, 0:64], vsrc)

                # attention for query blocks whose K/V coverage is complete
                if sc < 7:
                    hi = 4 * sc + 2
                    while done + 1 <= hi:
                        done += 1
                        attention(done)
                else:
                    # qb0's long serial chain starts early and its PE work is
                    # interleaved between the remaining query blocks
                    q0_scores()
                    attention(27)
                    attention(0)
                    for qb in range(28, NQB):
                        attention(qb)

    nc.compile()
    return nc


def kernel(x, Wq, Wk, Wv, Wo):
    from concourse import bass_utils

    x = np.asarray(x)
    B = x.shape[0]
    # xt chunks: [sc, p, c, 512]; hidden h = c*128 + p
    xt = np.ascontiguousarray(
        np.asarray(x)[0].T.astype(BF16).reshape(8, 128, 8, 512)
        .transpose(2, 1, 0, 3))

    def wlayout(W, rs, scale=1.0):
        # W[rs, :].T is [1024 (c p), 128 o] -> [p, c, o]
        wt = (np.asarray(W)[rs, :].T * scale).astype(BF16)
        return np.ascontiguousarray(wt.reshape(8, 128, OC).transpose(1, 0, 2))

    in_maps = []
    for d in range(N_CORES):
        rs = slice(OC * d, OC * (d + 1))
        in_maps.append({
            "xt": xt,
            "wqt": wlayout(Wq, rs, 0.125),
            "wkt": wlayout(Wk, rs),
            "wvt": wlayout(Wv, rs),
            "wot": np.ascontiguousarray(np.asarray(Wo)[:, rs].T.astype(BF16)),
        })

    if "nc" not in _CACHE:
        _CACHE["nc"] = _build()
    nc = _CACHE["nc"]

    res = bass_utils.run_bass_kernel_spmd(
        nc, in_maps, core_ids=list(range(N_CORES)),
        trace=bool(os.environ.get("KERNEL_TRACE")))
    global LAST_RESULTS
    LAST_RESULTS = res

    out = np.zeros((S, HIDDEN), np.float64)
    for r in res.results:
        out += r["partial"].astype(np.float64)
    return out.reshape(B, S, HIDDEN).astype(np.float32)


# revision 19
# speedup vs baseline: 1.0196x; 1.0196x over previous
"""Longformer attention TP-sharded Bass kernel for 8 NeuronCores (v2).

Sharding: tensor-parallel over heads. Core d owns heads 2d, 2d+1:
  - Wq/Wk/Wv rows [128d:128(d+1)]  (nn.Linear: q = x @ Wq.T)
  - Wo columns [128d:128(d+1)]
  Each core computes its heads' sparse (windowed+global) attention and a
  full-size out-proj partial; host sums the 8 partials (the "all-reduce").

v2 layout (all bf16 compute, fp32 PSUM accumulate):
  xT  [1024h, 4096s]   x transposed; DMA'd in 8 contiguous 1MB chunks
  qT/kT [128o, 4096s]  head dims on partitions (head A: 0-63, head B: 64-127)
  v   [128s, 32kb, 130] natural layout per key block: [vA | 1 | vB | 1]; the
                        ones columns make the PV matmul also emit the softmax
                        denominator.
  scoresT [k, q] per (qb, head) in one PSUM bank [128, 512]:
     [prev-block | next-block | diag-block | global-col strip(row 0, M=1 mm)]
  Global query row 0 (qb0) goes through M=1 strip matmuls so only the needed
  row is computed.  Masks are multiplicative 0/1 bf16 on exp(scores), 256
  cols wide, applied on the idle GpSimd engine.  Head A (PE rows 0-63) and
  head B (rows 64-127) score matmuls are emitted adjacently so the PE runs
  them concurrently (row-group tiling).
"""

import os
import numpy as np
import ml_dtypes

S = 4096
HIDDEN = 1024
N_CORES = 8
OC = 128          # out-proj contraction dims (head dims) per core = 2 heads x 64
NQB = S // 128    # 32 query/key blocks
BF16 = ml_dtypes.bfloat16

_CACHE = {}
LAST_RESULTS = None


def _masks_np():
    """Multiplicative masks [4, 128, 256] bf16, scoresT [k(part), q(free)]:
      0: interior qb: [keep f<=p (prev) | keep f>=p (next)]
      1: qb1:         [keep (f<=p)&(p>0) (kb0) | keep f>=p (kb2)]
      2: qb31:        [keep f<=p (kb30) | ones (diag)]
      3: qb0:         [keep (f>=p)|(f==0) (kb1) | ones (kb0)]
    """
    p = np.arange(128)[:, None]   # key index within block
    f = np.arange(128)[None, :]   # query index within block
    ones = np.ones((128, 128), bool)
    m_lo = (f <= p)
    m_lo_ng = m_lo & (p > 0)
    m_up = (f >= p)
    m_up0 = m_up | (f == 0)
    out = np.zeros((4, 128, 256), bool)
    out[0] = np.concatenate([m_lo, m_up], 1)
    out[1] = np.concatenate([m_lo_ng, m_up], 1)
    out[2] = np.concatenate([m_lo, ones], 1)
    out[3] = np.concatenate([m_up0, ones], 1)
    return out.astype(BF16)


def _band_for(qb):
    """[(key_block, col_offset)] band blocks for query block qb (qb >= 1)."""
    if qb == NQB - 1:
        return [(qb - 1, 0), (qb, 128)]
    return [(qb - 1, 0), (qb + 1, 128), (qb, 256)]


def _mask_cls(qb):
    if qb == 1:
        return 1
    if qb == NQB - 1:
        return 2
    return 0


def _build():
    import concourse.bass as bass
    import concourse.mybir as mybir
    import concourse.tile as tile
    from concourse import bacc

    f32 = mybir.dt.float32
    bf16 = mybir.dt.bfloat16
    Exp = mybir.ActivationFunctionType.Exp

    nc = bacc.Bacc("TRN2", target_bir_lowering=False, debug=False,
                   num_devices=N_CORES)

    # xt chunks: [sc][p][c][512] so each chunk is one contiguous 1MB transfer
    xt_d = nc.dram_tensor("xt", [8, 128, 8, 512], bf16, kind="ExternalInput").ap()
    # weights pre-arranged host-side: [p][c][o] contiguous
    wq_d = nc.dram_tensor("wqt", [128, 8, OC], bf16, kind="ExternalInput").ap()
    wk_d = nc.dram_tensor("wkt", [128, 8, OC], bf16, kind="ExternalInput").ap()
    wv_d = nc.dram_tensor("wvt", [128, 8, OC], bf16, kind="ExternalInput").ap()
    wo_d = nc.dram_tensor("wot", [OC, HIDDEN], bf16, kind="ExternalInput").ap()
    out_d = nc.dram_tensor("partial", [S, HIDDEN], bf16,
                           kind="ExternalOutput").ap()
    mask_d = nc.inline_tensor(
        np.ascontiguousarray(_masks_np().transpose(1, 0, 2)), name="masks").ap()
    id_d = nc.inline_tensor(np.eye(128, dtype=BF16), name="ident").ap()

    with tile.TileContext(nc) as tc:
        import contextlib
        with contextlib.ExitStack() as ctx:
            big = ctx.enter_context(tc.tile_pool(name="big", bufs=1))
            tmp = ctx.enter_context(tc.tile_pool(name="tmp", bufs=4))
            psS = ctx.enter_context(tc.tile_pool(name="psS", bufs=3, space="PSUM"))
            psP = ctx.enter_context(tc.tile_pool(name="psP", bufs=2, space="PSUM"))
            psT = ctx.enter_context(tc.tile_pool(name="psT", bufs=1, space="PSUM"))
            psO = ctx.enter_context(tc.tile_pool(name="psO", bufs=2, space="PSUM"))

            # ---- resident tensors ----
            xt_sb = big.tile([128, 8, 8, 512], bf16)  # x.T chunks [p, sc, c, s]
            qt_sb = big.tile([128, S], bf16)          # q.T (0.125 folded in Wq)
            kt_sb = big.tile([128, S], bf16)
            v_sb = big.tile([128, NQB, 130], bf16)    # [vA|1|vB|1] per key block
            outn_sb = big.tile([128, NQB, 128], bf16)  # attn out, natural [q, hd]
            outt_sb = big.tile([128, NQB, 128], bf16)  # transposed [hd, q]
            wq_sb = big.tile([128, 8, OC], bf16)
            wk_sb = big.tile([128, 8, OC], bf16)
            wv_sb = big.tile([128, 8, OC], bf16)
            wo_sb = big.tile([128, HIDDEN], bf16)
            mask_sb = big.tile([128, 4, 256], bf16)
            id_sb = big.tile([128, 128], bf16)

            scratch = big.tile([64, 64], bf16)    # never written: PE warmup fuel

            # ---- loads: wq + xt0 first so Q(sc0) starts asap ----
            nc.sync.dma_start(wq_sb, wq_d)
            nc.sync.dma_start(xt_sb[:, 0], xt_d[0])
            nc.sync.dma_start(wk_sb, wk_d)
            nc.sync.dma_start(wv_sb, wv_d)
            nc.sync.dma_start(xt_sb[:, 1], xt_d[1])
            nc.sync.dma_start(wo_sb, wo_d)
            nc.sync.dma_start(mask_sb, mask_d)
            nc.sync.dma_start(id_sb, id_d)
            nc.gpsimd.memset(v_sb[:, :, 64], 1.0)
            nc.gpsimd.memset(v_sb[:, :, 129], 1.0)
            nc.gpsimd.memset(scratch, 0.0)

            # keep the PE busy through the initial DMA wait so the HAM clock
            # gate is released (2.4 GHz) by the time real matmuls arrive
            psw = psT.tile([128, 128], f32, tag="tr", name="psw")
            for _ in range(90):
                nc.tensor.matmul(psw[0:64, 0:64], scratch, scratch,
                                 start=True, stop=True)

            q0_state = {}

            def q0_scores():
                # qb0: band [kb1 | kb0] + far strip (scores of q0 vs kb2..31)
                qsl = slice(0, 128)
                pss = [psS.tile([128, 512], f32, tag="s", name="pss")
                       for _ in range(2)]
                for kb, off in ((1, 0), (0, 128)):
                    for h in range(2):
                        bp = 64 * h
                        nc.tensor.matmul(
                            pss[h][:, off:off + 128],
                            kt_sb[bp:bp + 64, kb * 128:(kb + 1) * 128],
                            qt_sb[bp:bp + 64, qsl],
                            start=True, stop=True)
                for kb in range(2, NQB):
                    for h in range(2):
                        bp = 64 * h
                        nc.tensor.matmul(
                            pss[h][:, 254 + kb:255 + kb],
                            kt_sb[bp:bp + 64, kb * 128:(kb + 1) * 128],
                            qt_sb[bp:bp + 64, 0:1],
                            start=True, stop=True)
                probs = []
                for h in range(2):
                    # dedicated tag: these stay live across other qbs' rotations
                    pr = tmp.tile([128, 512], bf16, tag="probs0", bufs=2,
                                  name="pr0")
                    probs.append(pr)
                    nc.scalar.activation(pr[:, 0:286], pss[h][:, 0:286], Exp)
                    nc.gpsimd.tensor_mul(pr[:, 0:128], pr[:, 0:128],
                                         mask_sb[:, 3, 0:128])
                q0_state["probs"] = probs

            def attention(qb):
                qsl = slice(qb * 128, (qb + 1) * 128)
                pso = psP.tile([128, 512], f32, tag="pv", name="pso")
                probs = [None, None]
                if qb > 0:
                    band = _band_for(qb)
                    pss = [psS.tile([128, 512], f32, tag="s", name="pss")
                           for _ in range(2)]
                    # head-interleaved band matmuls: A rows 0-63, B rows 64-127
                    # run concurrently in the PE array (distinct row groups)
                    for kb, off in band:
                        for h in range(2):
                            bp = 64 * h
                            nc.tensor.matmul(
                                pss[h][:, off:off + 128],
                                kt_sb[bp:bp + 64, kb * 128:(kb + 1) * 128],
                                qt_sb[bp:bp + 64, qsl],
                                start=True, stop=True)
                    # global key-0 column strip: M=1, row 0 of cols 384:512
                    for h in range(2):
                        bp = 64 * h
                        nc.tensor.matmul(
                            pss[h][0:1, 384:512],
                            kt_sb[bp:bp + 64, 0:1],
                            qt_sb[bp:bp + 64, qsl],
                            start=True, stop=True)
                    cls = _mask_cls(qb)
                    for h in range(2):
                        pr = tmp.tile([128, 512], bf16, tag="probs", name="pr")
                        probs[h] = pr
                        nc.scalar.activation(pr, pss[h], Exp)
                        nc.gpsimd.tensor_mul(pr[:, 0:256], pr[:, 0:256],
                                             mask_sb[:, cls, :])
                    # PV: probs stationary, v moving; ones cols give denoms.
                    # diag + key-0 strip first: they only need exp, not the
                    # mask, so the PE resumes sooner after the activation.
                    for h in range(2):
                        hsl = slice(65 * h, 65 * h + 65)
                        vsl = slice(65 * h, 65 * h + 65)
                        diag = band[-1]
                        nc.tensor.matmul(
                            pso[:, hsl], probs[h][:, diag[1]:diag[1] + 128],
                            v_sb[:, diag[0], vsl],
                            start=True, stop=False, skip_group_check=True)
                        # key-0 contribution: K=1 outer product
                        nc.tensor.matmul(
                            pso[:, hsl], probs[h][0:1, 384:512],
                            v_sb[0:1, 0, vsl],
                            start=False, stop=False, skip_group_check=True)
                        for j, (kb, off) in enumerate(band[:-1]):
                            nc.tensor.matmul(
                                pso[:, hsl], probs[h][:, off:off + 128],
                                v_sb[:, kb, vsl],
                                start=False, stop=(j == len(band) - 2),
                                skip_group_check=True)
                else:
                    probs = q0_state["probs"]
                    for h in range(2):
                        hsl = slice(65 * h, 65 * h + 65)
                        vsl = slice(65 * h, 65 * h + 65)
                        for j, (kb, off) in enumerate(((1, 0), (0, 128))):
                            nc.tensor.matmul(
                                pso[:, hsl], probs[h][:, off:off + 128],
                                v_sb[:, kb, vsl],
                                start=(j == 0), stop=False,
                                skip_group_check=True)
                        for kb in range(2, NQB):
                            nc.tensor.matmul(
                                pso[0:1, hsl],
                                probs[h][:, 254 + kb:255 + kb],
                                v_sb[:, kb, vsl],
                                start=False, stop=(kb == NQB - 1),
                                skip_group_check=True)

                # normalize + write outn
                recip = tmp.tile([128, 2], f32, tag="recip", name="recip")
                pso_h = pso[:, 0:130].rearrange("p (h c) -> p h c", h=2)
                nc.vector.reciprocal(recip, pso_h[:, :, 64])
                for h in range(2):
                    nc.vector.tensor_scalar_mul(
                        outn_sb[:, qb, 64 * h:64 * h + 64],
                        pso[:, 65 * h:65 * h + 64], recip[:, h:h + 1])

                # transpose -> out-proj -> stage -> DMA
                pstr = psT.tile([128, 128], bf16, tag="tr", name="pstr")
                nc.tensor.transpose(pstr, outn_sb[:, qb, :], id_sb)
                nc.vector.tensor_copy(outt_sb[:, qb, :], pstr)
                stage = tmp.tile([128, HIDDEN], bf16, tag="stage", name="stage")
                for oc in range(2):
                    psp = psO.tile([128, 512], f32, tag="o", name="psp")
                    nc.tensor.matmul(psp, outt_sb[:, qb, :],
                                     wo_sb[:, oc * 512:(oc + 1) * 512],
                                     start=True, stop=True)
                    if oc == 0:
                        nc.vector.tensor_copy(
                            stage[:, oc * 512:(oc + 1) * 512], psp)
                    else:
                        nc.scalar.copy(stage[:, oc * 512:(oc + 1) * 512], psp)
                nc.sync.dma_start(out_d[qb * 128:(qb + 1) * 128, :], stage)

            # ---- projections interleaved with attention ----
            done = 0
            for sc in range(8):
                if 2 <= sc + 1 <= 7:
                    nc.sync.dma_start(xt_sb[:, sc + 1], xt_d[sc + 1])
                ssl = slice(sc * 512, (sc + 1) * 512)

                psq = psS.tile([128, 512], f32, tag="s", name="psq")
                for hc in range(8):
                    nc.tensor.matmul(psq, wq_sb[:, hc, :], xt_sb[:, sc, hc],
                                     start=(hc == 0), stop=(hc == 7))
                nc.vector.tensor_copy(qt_sb[:, ssl], psq)

                psk = psS.tile([128, 512], f32, tag="s", name="psk")
                for hc in range(8):
                    nc.tensor.matmul(psk, wk_sb[:, hc, :], xt_sb[:, sc, hc],
                                     start=(hc == 0), stop=(hc == 7))
                nc.vector.tensor_copy(kt_sb[:, ssl], psk)

                psv = psS.tile([128, 512], f32, tag="s", name="psv")
                for b in range(4):
                    bsl = slice(b * 128, b * 128 + 128)
                    for hc in range(8):
                        nc.tensor.matmul(psv[:, b * 128:b * 128 + 128],
                                         xt_sb[:, sc, hc, bsl], wv_sb[:, hc, :],
                                         start=(hc == 0), stop=(hc == 7),
                                         skip_group_check=True)
                # one strided copy for all 4 blocks x 2 heads
                vdst = v_sb[:, sc * 4:sc * 4 + 4, :].rearrange(
                    "p b (h c) -> p b h c", h=2)
                vsrc = psv.rearrange("p (b h c) -> p b h c", b=4, h=2)
                nc.vector.tensor_copy(vdst[:, :, :, 0:64], vsrc)

                # attention for query blocks whose K/V coverage is complete
                if sc < 7:
                    hi = 4 * sc + 2
                    while done + 1 <= hi:
                        done += 1
                        attention(done)
                else:
                    # qb0's long serial chain starts early and its PE work is
                    # interleaved between the remaining query blocks
                    q0_scores()
                    attention(27)
                    attention(0)
                    for qb in range(28, NQB):
                        attention(qb)

    nc.compile()
    return nc


def kernel(x, Wq, Wk, Wv, Wo):
    from concourse import bass_utils

    x = np.asarray(x)
    B = x.shape[0]
    # xt chunks: [sc, p, c, 512]; hidden h = c*128 + p
    xt = np.ascontiguousarray(
        np.asarray(x)[0].T.astype(BF16).reshape(8, 128, 8, 512)
        .transpose(2, 1, 0, 3))

    def wlayout(W, rs, scale=1.0):
        # W[rs, :].T is [1024 (c p), 128 o] -> [p, c, o]
        wt = (np.asarray(W)[rs, :].T * scale).astype(BF16)
        return np.ascontiguousarray(wt.reshape(8, 128, OC).transpose(1, 0, 2))

    in_maps = []
    for d in range(N_CORES):
        rs = slice(OC * d, OC * (d + 1))
        in_maps.append({
            "xt": xt,
            "wqt": wlayout(Wq, rs, 0.125),
            "wkt": wlayout(Wk, rs),
            "wvt": wlayout(Wv, rs),
            "wot": np.ascontiguousarray(np.asarray(Wo)[:, rs].T.astype(BF16)),
        })

    if "nc" not in _CACHE:
        _CACHE["nc"] = _build()
    nc = _CACHE["nc"]

    res = bass_utils.run_bass_kernel_spmd(
        nc, in_maps, core_ids=list(range(N_CORES)),
        trace=bool(os.environ.get("KERNEL_TRACE")))
    global LAST_RESULTS
    LAST_RESULTS = res

    out = np.zeros((S, HIDDEN), np.float64)
    for r in res.results:
        out += r["partial"].astype(np.float64)
    return out.reshape(B, S, HIDDEN).astype(np.float32)


# revision 20
# speedup vs baseline: 1.0718x; 1.0512x over previous
"""Longformer attention TP-sharded Bass kernel for 8 NeuronCores (v2).

Sharding: tensor-parallel over heads. Core d owns heads 2d, 2d+1:
  - Wq/Wk/Wv rows [128d:128(d+1)]  (nn.Linear: q = x @ Wq.T)
  - Wo columns [128d:128(d+1)]
  Each core computes its heads' sparse (windowed+global) attention and a
  full-size out-proj partial; host sums the 8 partials (the "all-reduce").

v2 layout (all bf16 compute, fp32 PSUM accumulate):
  xT  [1024h, 4096s]   x transposed; DMA'd in 8 contiguous 1MB chunks
  qT/kT [128o, 4096s]  head dims on partitions (head A: 0-63, head B: 64-127)
  v   [128s, 32kb, 130] natural layout per key block: [vA | 1 | vB | 1]; the
                        ones columns make the PV matmul also emit the softmax
                        denominator.
  scoresT [k, q] per (qb, head) in one PSUM bank [128, 512]:
     [prev-block | next-block | diag-block | global-col strip(row 0, M=1 mm)]
  Global query row 0 (qb0) goes through M=1 strip matmuls so only the needed
  row is computed.  Masks are multiplicative 0/1 bf16 on exp(scores), 256
  cols wide, applied on the idle GpSimd engine.  Head A (PE rows 0-63) and
  head B (rows 64-127) score matmuls are emitted adjacently so the PE runs
  them concurrently (row-group tiling).
"""

import os
import numpy as np
import ml_dtypes

S = 4096
HIDDEN = 1024
N_CORES = 8
OC = 128          # out-proj contraction dims (head dims) per core = 2 heads x 64
NQB = S // 128    # 32 query/key blocks
BF16 = ml_dtypes.bfloat16

_CACHE = {}
LAST_RESULTS = None


def _masks_np():
    """Multiplicative masks [4, 128, 256] bf16, scoresT [k(part), q(free)]:
      0: interior qb: [keep f<=p (prev) | keep f>=p (next)]
      1: qb1:         [keep (f<=p)&(p>0) (kb0) | keep f>=p (kb2)]
      2: qb31:        [keep f<=p (kb30) | ones (diag)]
      3: qb0:         [keep (f>=p)|(f==0) (kb1) | ones (kb0)]
    """
    p = np.arange(128)[:, None]   # key index within block
    f = np.arange(128)[None, :]   # query index within block
    ones = np.ones((128, 128), bool)
    m_lo = (f <= p)
    m_lo_ng = m_lo & (p > 0)
    m_up = (f >= p)
    m_up0 = m_up | (f == 0)
    out = np.zeros((4, 128, 256), bool)
    out[0] = np.concatenate([m_lo, m_up], 1)
    out[1] = np.concatenate([m_lo_ng, m_up], 1)
    out[2] = np.concatenate([m_lo, ones], 1)
    out[3] = np.concatenate([m_up0, ones], 1)
    return out.astype(BF16)


def _band_for(qb):
    """[(key_block, col_offset)] band blocks for query block qb (qb >= 1)."""
    if qb == NQB - 1:
        return [(qb - 1, 0), (qb, 128)]
    return [(qb - 1, 0), (qb + 1, 128), (qb, 256)]


def _mask_cls(qb):
    if qb == 1:
        return 1
    if qb == NQB - 1:
        return 2
    return 0


def _build():
    import concourse.bass as bass
    import concourse.mybir as mybir
    import concourse.tile as tile
    from concourse import bacc

    f32 = mybir.dt.float32
    bf16 = mybir.dt.bfloat16
    Exp = mybir.ActivationFunctionType.Exp

    nc = bacc.Bacc("TRN2", target_bir_lowering=False, debug=False,
                   num_devices=N_CORES)

    # xt chunks: [sc][p][c][512] so each chunk is one contiguous 1MB transfer
    xt_d = nc.dram_tensor("xt", [8, 128, 8, 512], bf16, kind="ExternalInput").ap()
    # weights pre-arranged host-side: [p][c][o] contiguous
    wq_d = nc.dram_tensor("wqt", [128, 8, OC], bf16, kind="ExternalInput").ap()
    wk_d = nc.dram_tensor("wkt", [128, 8, OC], bf16, kind="ExternalInput").ap()
    wv_d = nc.dram_tensor("wvt", [128, 8, OC], bf16, kind="ExternalInput").ap()
    wo_d = nc.dram_tensor("wot", [OC, HIDDEN], bf16, kind="ExternalInput").ap()
    out_d = nc.dram_tensor("partial", [S, HIDDEN], bf16,
                           kind="ExternalOutput").ap()
    mask_d = nc.inline_tensor(
        np.ascontiguousarray(_masks_np().transpose(1, 0, 2)), name="masks").ap()
    id_d = nc.inline_tensor(np.eye(128, dtype=BF16), name="ident").ap()

    with tile.TileContext(nc) as tc:
        import contextlib
        with contextlib.ExitStack() as ctx:
            big = ctx.enter_context(tc.tile_pool(name="big", bufs=1))
            tmp = ctx.enter_context(tc.tile_pool(name="tmp", bufs=4))
            psS = ctx.enter_context(tc.tile_pool(name="psS", bufs=3, space="PSUM"))
            psP = ctx.enter_context(tc.tile_pool(name="psP", bufs=2, space="PSUM"))
            psT = ctx.enter_context(tc.tile_pool(name="psT", bufs=1, space="PSUM"))
            psO = ctx.enter_context(tc.tile_pool(name="psO", bufs=2, space="PSUM"))

            # ---- resident tensors ----
            xt_sb = big.tile([128, 8, 8, 512], bf16)  # x.T chunks [p, sc, c, s]
            qt_sb = big.tile([128, S], bf16)          # q.T (0.125 folded in Wq)
            kt_sb = big.tile([128, S], bf16)
            v_sb = big.tile([128, NQB, 130], bf16)    # [vA|1|vB|1] per key block
            outn_sb = big.tile([128, NQB, 128], bf16)  # attn out, natural [q, hd]
            outt_sb = big.tile([128, NQB, 128], bf16)  # transposed [hd, q]
            wq_sb = big.tile([128, 8, OC], bf16)
            wk_sb = big.tile([128, 8, OC], bf16)
            wv_sb = big.tile([128, 8, OC], bf16)
            wo_sb = big.tile([128, HIDDEN], bf16)
            mask_sb = big.tile([128, 4, 256], bf16)
            id_sb = big.tile([128, 128], bf16)

            scratch = big.tile([64, 64], bf16)    # never written: PE warmup fuel

            # ---- loads: wq + xt0 first so Q(sc0) starts asap ----
            nc.sync.dma_start(wq_sb, wq_d)
            nc.sync.dma_start(xt_sb[:, 0], xt_d[0])
            nc.sync.dma_start(wk_sb, wk_d)
            nc.sync.dma_start(wv_sb, wv_d)
            nc.sync.dma_start(xt_sb[:, 1], xt_d[1])
            nc.sync.dma_start(wo_sb, wo_d)
            nc.sync.dma_start(mask_sb, mask_d)
            nc.sync.dma_start(id_sb, id_d)
            nc.gpsimd.memset(v_sb[:, :, 64], 1.0)
            nc.gpsimd.memset(v_sb[:, :, 129], 1.0)
            nc.gpsimd.memset(scratch, 0.0)

            # keep the PE busy through the initial DMA wait so the HAM clock
            # gate is released (2.4 GHz) by the time real matmuls arrive
            psw = psT.tile([128, 128], f32, tag="tr", name="psw")
            for _ in range(90):
                nc.tensor.matmul(psw[0:64, 0:64], scratch, scratch,
                                 start=True, stop=True)

            q0_state = {}

            def q0_scores():
                # qb0: band [kb1 | kb0] + far strip (scores of q0 vs kb2..31)
                qsl = slice(0, 128)
                pss = [psS.tile([128, 512], f32, tag="s", name="pss")
                       for _ in range(2)]
                for kb, off in ((1, 0), (0, 128)):
                    for h in range(2):
                        bp = 64 * h
                        nc.tensor.matmul(
                            pss[h][:, off:off + 128],
                            kt_sb[bp:bp + 64, kb * 128:(kb + 1) * 128],
                            qt_sb[bp:bp + 64, qsl],
                            start=True, stop=True)
                for kb in range(2, NQB):
                    for h in range(2):
                        bp = 64 * h
                        nc.tensor.matmul(
                            pss[h][:, 254 + kb:255 + kb],
                            kt_sb[bp:bp + 64, kb * 128:(kb + 1) * 128],
                            qt_sb[bp:bp + 64, 0:1],
                            start=True, stop=True)
                probs = []
                for h in range(2):
                    # dedicated tag: these stay live across other qbs' rotations
                    pr = tmp.tile([128, 512], bf16, tag="probs0", bufs=2,
                                  name="pr0")
                    probs.append(pr)
                    nc.scalar.activation(pr[:, 0:286], pss[h][:, 0:286], Exp)
                    eng = nc.vector if h == 0 else nc.gpsimd
                    eng.tensor_mul(pr[:, 0:128], pr[:, 0:128],
                                   mask_sb[:, 3, 0:128])
                q0_state["probs"] = probs

            def attention(qb):
                qsl = slice(qb * 128, (qb + 1) * 128)
                pso = psP.tile([128, 512], f32, tag="pv", name="pso")
                probs = [None, None]
                if qb > 0:
                    band = _band_for(qb)
                    pss = [psS.tile([128, 512], f32, tag="s", name="pss")
                           for _ in range(2)]
                    # head-interleaved band matmuls: A rows 0-63, B rows 64-127
                    # run concurrently in the PE array (distinct row groups)
                    for kb, off in band:
                        for h in range(2):
                            bp = 64 * h
                            nc.tensor.matmul(
                                pss[h][:, off:off + 128],
                                kt_sb[bp:bp + 64, kb * 128:(kb + 1) * 128],
                                qt_sb[bp:bp + 64, qsl],
                                start=True, stop=True)
                    # global key-0 column strip: M=1, row 0 of cols 384:512
                    for h in range(2):
                        bp = 64 * h
                        nc.tensor.matmul(
                            pss[h][0:1, 384:512],
                            kt_sb[bp:bp + 64, 0:1],
                            qt_sb[bp:bp + 64, qsl],
                            start=True, stop=True)
                    cls = _mask_cls(qb)
                    for h in range(2):
                        pr = tmp.tile([128, 512], bf16, tag="probs", name="pr")
                        probs[h] = pr
                        nc.scalar.activation(pr, pss[h], Exp)
                        eng = nc.vector if h == 0 else nc.gpsimd
                        eng.tensor_mul(pr[:, 0:256], pr[:, 0:256],
                                       mask_sb[:, cls, :])
                    # PV: probs stationary, v moving; ones cols give denoms.
                    # diag + key-0 strip first: they only need exp, not the
                    # mask, so the PE resumes sooner after the activation.
                    for h in range(2):
                        hsl = slice(65 * h, 65 * h + 65)
                        vsl = slice(65 * h, 65 * h + 65)
                        diag = band[-1]
                        nc.tensor.matmul(
                            pso[:, hsl], probs[h][:, diag[1]:diag[1] + 128],
                            v_sb[:, diag[0], vsl],
                            start=True, stop=False, skip_group_check=True)
                        # key-0 contribution: K=1 outer product
                        nc.tensor.matmul(
                            pso[:, hsl], probs[h][0:1, 384:512],
                            v_sb[0:1, 0, vsl],
                            start=False, stop=False, skip_group_check=True)
                        for j, (kb, off) in enumerate(band[:-1]):
                            nc.tensor.matmul(
                                pso[:, hsl], probs[h][:, off:off + 128],
                                v_sb[:, kb, vsl],
                                start=False, stop=(j == len(band) - 2),
                                skip_group_check=True)
                else:
                    probs = q0_state["probs"]
                    for h in range(2):
                        hsl = slice(65 * h, 65 * h + 65)
                        vsl = slice(65 * h, 65 * h + 65)
                        for j, (kb, off) in enumerate(((1, 0), (0, 128))):
                            nc.tensor.matmul(
                                pso[:, hsl], probs[h][:, off:off + 128],
                                v_sb[:, kb, vsl],
                                start=(j == 0), stop=False,
                                skip_group_check=True)
                        for kb in range(2, NQB):
                            nc.tensor.matmul(
                                pso[0:1, hsl],
                                probs[h][:, 254 + kb:255 + kb],
                                v_sb[:, kb, vsl],
                                start=False, stop=(kb == NQB - 1),
                                skip_group_check=True)

                # normalize + write outn
                recip = tmp.tile([128, 2], f32, tag="recip", name="recip")
                pso_h = pso[:, 0:130].rearrange("p (h c) -> p h c", h=2)
                nc.vector.reciprocal(recip, pso_h[:, :, 64])
                for h in range(2):
                    nc.vector.tensor_scalar_mul(
                        outn_sb[:, qb, 64 * h:64 * h + 64],
                        pso[:, 65 * h:65 * h + 64], recip[:, h:h + 1])

                # transpose -> out-proj -> stage -> DMA
                pstr = psT.tile([128, 128], bf16, tag="tr", name="pstr")
                nc.tensor.transpose(pstr, outn_sb[:, qb, :], id_sb)
                nc.vector.tensor_copy(outt_sb[:, qb, :], pstr)
                stage = tmp.tile([128, HIDDEN], bf16, tag="stage", name="stage")
                for oc in range(2):
                    psp = psO.tile([128, 512], f32, tag="o", name="psp")
                    nc.tensor.matmul(psp, outt_sb[:, qb, :],
                                     wo_sb[:, oc * 512:(oc + 1) * 512],
                                     start=True, stop=True)
                    if oc == 0:
                        nc.vector.tensor_copy(
                            stage[:, oc * 512:(oc + 1) * 512], psp)
                    else:
                        nc.scalar.copy(stage[:, oc * 512:(oc + 1) * 512], psp)
                nc.sync.dma_start(out_d[qb * 128:(qb + 1) * 128, :], stage)

            # ---- projections interleaved with attention ----
            done = 0
            for sc in range(8):
                if 2 <= sc + 1 <= 7:
                    nc.sync.dma_start(xt_sb[:, sc + 1], xt_d[sc + 1])
                ssl = slice(sc * 512, (sc + 1) * 512)

                psq = psS.tile([128, 512], f32, tag="s", name="psq")
                for hc in range(8):
                    nc.tensor.matmul(psq, wq_sb[:, hc, :], xt_sb[:, sc, hc],
                                     start=(hc == 0), stop=(hc == 7))
                nc.vector.tensor_copy(qt_sb[:, ssl], psq)

                psk = psS.tile([128, 512], f32, tag="s", name="psk")
                for hc in range(8):
                    nc.tensor.matmul(psk, wk_sb[:, hc, :], xt_sb[:, sc, hc],
                                     start=(hc == 0), stop=(hc == 7))
                nc.scalar.copy(kt_sb[:, ssl], psk)

                psv = psS.tile([128, 512], f32, tag="s", name="psv")
                for b in range(4):
                    bsl = slice(b * 128, b * 128 + 128)
                    for hc in range(8):
                        nc.tensor.matmul(psv[:, b * 128:b * 128 + 128],
                                         xt_sb[:, sc, hc, bsl], wv_sb[:, hc, :],
                                         start=(hc == 0), stop=(hc == 7),
                                         skip_group_check=True)
                # one strided copy for all 4 blocks x 2 heads
                vdst = v_sb[:, sc * 4:sc * 4 + 4, :].rearrange(
                    "p b (h c) -> p b h c", h=2)
                vsrc = psv.rearrange("p (b h c) -> p b h c", b=4, h=2)
                nc.vector.tensor_copy(vdst[:, :, :, 0:64], vsrc)

                # attention for query blocks whose K/V coverage is complete
                if sc < 7:
                    hi = 4 * sc + 2
                    while done + 1 <= hi:
                        done += 1
                        attention(done)
                else:
                    # qb0's long serial chain starts early and its PE work is
                    # interleaved between the remaining query blocks
                    q0_scores()
                    attention(27)
                    attention(0)
                    for qb in range(28, NQB):
                        attention(qb)

    nc.compile()
    return nc


def kernel(x, Wq, Wk, Wv, Wo):
    from concourse import bass_utils

    x = np.asarray(x)
    B = x.shape[0]
    # xt chunks: [sc, p, c, 512]; hidden h = c*128 + p
    xt = np.ascontiguousarray(
        np.asarray(x)[0].T.astype(BF16).reshape(8, 128, 8, 512)
        .transpose(2, 1, 0, 3))

    def wlayout(W, rs, scale=1.0):
        # W[rs, :].T is [1024 (c p), 128 o] -> [p, c, o]
        wt = (np.asarray(W)[rs, :].T * scale).astype(BF16)
        return np.ascontiguousarray(wt.reshape(8, 128, OC).transpose(1, 0, 2))

    in_maps = []
    for d in range(N_CORES):
        rs = slice(OC * d, OC * (d + 1))
        in_maps.append({
            "xt": xt,
            "wqt": wlayout(Wq, rs, 0.125),
            "wkt": wlayout(Wk, rs),
            "wvt": wlayout(Wv, rs),
            "wot": np.ascontiguousarray(np.asarray(Wo)[:, rs].T.astype(BF16)),
        })

    if "nc" not in _CACHE:
        _CACHE["nc"] = _build()
    nc = _CACHE["nc"]

    res = bass_utils.run_bass_kernel_spmd(
        nc, in_maps, core_ids=list(range(N_CORES)),
        trace=bool(os.environ.get("KERNEL_TRACE")))
    global LAST_RESULTS
    LAST_RESULTS = res

    out = np.zeros((S, HIDDEN), np.float64)
    for r in res.results:
        out += r["partial"].astype(np.float64)
    return out.reshape(B, S, HIDDEN).astype(np.float32)


# revision 21
# speedup vs baseline: 1.0935x; 1.0203x over previous
"""Longformer attention TP-sharded Bass kernel for 8 NeuronCores (v2).

Sharding: tensor-parallel over heads. Core d owns heads 2d, 2d+1:
  - Wq/Wk/Wv rows [128d:128(d+1)]  (nn.Linear: q = x @ Wq.T)
  - Wo columns [128d:128(d+1)]
  Each core computes its heads' sparse (windowed+global) attention and a
  full-size out-proj partial; host sums the 8 partials (the "all-reduce").

v2 layout (all bf16 compute, fp32 PSUM accumulate):
  xT  [1024h, 4096s]   x transposed; DMA'd in 8 contiguous 1MB chunks
  qT/kT [128o, 4096s]  head dims on partitions (head A: 0-63, head B: 64-127)
  v   [128s, 32kb, 130] natural layout per key block: [vA | 1 | vB | 1]; the
                        ones columns make the PV matmul also emit the softmax
                        denominator.
  scoresT [k, q] per (qb, head) in one PSUM bank [128, 512]:
     [prev-block | next-block | diag-block | global-col strip(row 0, M=1 mm)]
  Global query row 0 (qb0) goes through M=1 strip matmuls so only the needed
  row is computed.  Masks are multiplicative 0/1 bf16 on exp(scores), 256
  cols wide, applied on the idle GpSimd engine.  Head A (PE rows 0-63) and
  head B (rows 64-127) score matmuls are emitted adjacently so the PE runs
  them concurrently (row-group tiling).
"""

import os
import numpy as np
import ml_dtypes

S = 4096
HIDDEN = 1024
N_CORES = 8
OC = 128          # out-proj contraction dims (head dims) per core = 2 heads x 64
NQB = S // 128    # 32 query/key blocks
BF16 = ml_dtypes.bfloat16

_CACHE = {}
LAST_RESULTS = None


def _masks_np():
    """Multiplicative masks [4, 128, 256] bf16, scoresT [k(part), q(free)]:
      0: interior qb: [keep f<=p (prev) | keep f>=p (next)]
      1: qb1:         [keep (f<=p)&(p>0) (kb0) | keep f>=p (kb2)]
      2: qb31:        [keep f<=p (kb30) | ones (diag)]
      3: qb0:         [keep (f>=p)|(f==0) (kb1) | ones (kb0)]
    """
    p = np.arange(128)[:, None]   # key index within block
    f = np.arange(128)[None, :]   # query index within block
    ones = np.ones((128, 128), bool)
    m_lo = (f <= p)
    m_lo_ng = m_lo & (p > 0)
    m_up = (f >= p)
    m_up0 = m_up | (f == 0)
    out = np.zeros((4, 128, 256), bool)
    out[0] = np.concatenate([m_lo, m_up], 1)
    out[1] = np.concatenate([m_lo_ng, m_up], 1)
    out[2] = np.concatenate([m_lo, ones], 1)
    out[3] = np.concatenate([m_up0, ones], 1)
    return out.astype(BF16)


def _band_for(qb):
    """[(key_block, col_offset)] band blocks for query block qb (qb >= 1)."""
    if qb == NQB - 1:
        return [(qb - 1, 0), (qb, 128)]
    return [(qb - 1, 0), (qb + 1, 128), (qb, 256)]


def _mask_cls(qb):
    if qb == 1:
        return 1
    if qb == NQB - 1:
        return 2
    return 0


def _build():
    import concourse.bass as bass
    import concourse.mybir as mybir
    import concourse.tile as tile
    from concourse import bacc

    f32 = mybir.dt.float32
    bf16 = mybir.dt.bfloat16
    Exp = mybir.ActivationFunctionType.Exp

    nc = bacc.Bacc("TRN2", target_bir_lowering=False, debug=False,
                   num_devices=N_CORES)

    # xt chunks: [sc][p][c][512] so each chunk is one contiguous 1MB transfer
    xt_d = nc.dram_tensor("xt", [8, 128, 8, 512], bf16, kind="ExternalInput").ap()
    # weights pre-arranged host-side: [p][c][o] contiguous
    wq_d = nc.dram_tensor("wqt", [128, 8, OC], bf16, kind="ExternalInput").ap()
    wk_d = nc.dram_tensor("wkt", [128, 8, OC], bf16, kind="ExternalInput").ap()
    wv_d = nc.dram_tensor("wvt", [128, 8, OC], bf16, kind="ExternalInput").ap()
    wo_d = nc.dram_tensor("wot", [OC, HIDDEN], bf16, kind="ExternalInput").ap()
    out_d = nc.dram_tensor("partial", [S, HIDDEN], bf16,
                           kind="ExternalOutput").ap()
    mask_d = nc.inline_tensor(
        np.ascontiguousarray(_masks_np().transpose(1, 0, 2)), name="masks").ap()
    id_d = nc.inline_tensor(np.eye(128, dtype=BF16), name="ident").ap()

    with tile.TileContext(nc) as tc:
        import contextlib
        with contextlib.ExitStack() as ctx:
            big = ctx.enter_context(tc.tile_pool(name="big", bufs=1))
            tmp = ctx.enter_context(tc.tile_pool(name="tmp", bufs=4))
            psS = ctx.enter_context(tc.tile_pool(name="psS", bufs=3, space="PSUM"))
            psP = ctx.enter_context(tc.tile_pool(name="psP", bufs=2, space="PSUM"))
            psT = ctx.enter_context(tc.tile_pool(name="psT", bufs=1, space="PSUM"))
            psO = ctx.enter_context(tc.tile_pool(name="psO", bufs=2, space="PSUM"))

            # ---- resident tensors ----
            xt_sb = big.tile([128, 8, 8, 512], bf16)  # x.T chunks [p, sc, c, s]
            qt_sb = big.tile([128, S], bf16)          # q.T (0.125 folded in Wq)
            kt_sb = big.tile([128, S], bf16)
            v_sb = big.tile([128, NQB, 130], bf16)    # [vA|1|vB|1] per key block
            outn_sb = big.tile([128, NQB, 128], bf16)  # attn out, natural [q, hd]
            outt_sb = big.tile([128, NQB, 128], bf16)  # transposed [hd, q]
            wq_sb = big.tile([128, 8, OC], bf16)
            wk_sb = big.tile([128, 8, OC], bf16)
            wv_sb = big.tile([128, 8, OC], bf16)
            wo_sb = big.tile([128, HIDDEN], bf16)
            mask_sb = big.tile([128, 4, 256], bf16)
            id_sb = big.tile([128, 128], bf16)

            scratch = big.tile([64, 64], bf16)    # never written: PE warmup fuel

            # ---- loads: wq + xt0 first so Q(sc0) starts asap ----
            nc.sync.dma_start(wq_sb, wq_d)
            nc.sync.dma_start(xt_sb[:, 0], xt_d[0])
            nc.sync.dma_start(wk_sb, wk_d)
            nc.sync.dma_start(wv_sb, wv_d)
            nc.sync.dma_start(xt_sb[:, 1], xt_d[1])
            nc.sync.dma_start(wo_sb, wo_d)
            nc.sync.dma_start(mask_sb, mask_d)
            nc.sync.dma_start(id_sb, id_d)
            nc.gpsimd.memset(v_sb[:, :, 64], 1.0)
            nc.gpsimd.memset(v_sb[:, :, 129], 1.0)
            nc.gpsimd.memset(scratch, 0.0)

            # keep the PE busy through the initial DMA wait so the HAM clock
            # gate is released (2.4 GHz) by the time real matmuls arrive
            psw = psT.tile([128, 128], f32, tag="tr", name="psw")
            for _ in range(30):
                nc.tensor.matmul(psw[0:64, 0:64], scratch, scratch,
                                 start=True, stop=True)

            q0_state = {}

            def q0_scores():
                # qb0: band [kb1 | kb0] + far strip (scores of q0 vs kb2..31)
                qsl = slice(0, 128)
                pss = [psS.tile([128, 512], f32, tag="s", name="pss")
                       for _ in range(2)]
                for kb, off in ((1, 0), (0, 128)):
                    for h in range(2):
                        bp = 64 * h
                        nc.tensor.matmul(
                            pss[h][:, off:off + 128],
                            kt_sb[bp:bp + 64, kb * 128:(kb + 1) * 128],
                            qt_sb[bp:bp + 64, qsl],
                            start=True, stop=True)
                for kb in range(2, NQB):
                    for h in range(2):
                        bp = 64 * h
                        nc.tensor.matmul(
                            pss[h][:, 254 + kb:255 + kb],
                            kt_sb[bp:bp + 64, kb * 128:(kb + 1) * 128],
                            qt_sb[bp:bp + 64, 0:1],
                            start=True, stop=True)
                probs = []
                for h in range(2):
                    # dedicated tag: these stay live across other qbs' rotations
                    pr = tmp.tile([128, 512], bf16, tag="probs0", bufs=2,
                                  name="pr0")
                    probs.append(pr)
                    nc.scalar.activation(pr[:, 0:286], pss[h][:, 0:286], Exp)
                    eng = nc.vector if h == 0 else nc.gpsimd
                    eng.tensor_mul(pr[:, 0:128], pr[:, 0:128],
                                   mask_sb[:, 3, 0:128])
                q0_state["probs"] = probs

            def attention(qb):
                qsl = slice(qb * 128, (qb + 1) * 128)
                pso = psP.tile([128, 512], f32, tag="pv", name="pso")
                probs = [None, None]
                if qb > 0:
                    band = _band_for(qb)
                    pss = [psS.tile([128, 512], f32, tag="s", name="pss")
                           for _ in range(2)]
                    # head-interleaved band matmuls: A rows 0-63, B rows 64-127
                    # run concurrently in the PE array (distinct row groups)
                    for kb, off in band:
                        for h in range(2):
                            bp = 64 * h
                            nc.tensor.matmul(
                                pss[h][:, off:off + 128],
                                kt_sb[bp:bp + 64, kb * 128:(kb + 1) * 128],
                                qt_sb[bp:bp + 64, qsl],
                                start=True, stop=True)
                    # global key-0 column strip: M=1, row 0 of cols 384:512
                    for h in range(2):
                        bp = 64 * h
                        nc.tensor.matmul(
                            pss[h][0:1, 384:512],
                            kt_sb[bp:bp + 64, 0:1],
                            qt_sb[bp:bp + 64, qsl],
                            start=True, stop=True)
                    cls = _mask_cls(qb)
                    for h in range(2):
                        pr = tmp.tile([128, 512], bf16, tag="probs", name="pr")
                        probs[h] = pr
                        nc.scalar.activation(pr, pss[h], Exp)
                        eng = nc.vector if h == 0 else nc.gpsimd
                        eng.tensor_mul(pr[:, 0:256], pr[:, 0:256],
                                       mask_sb[:, cls, :])
                    # PV: probs stationary, v moving; ones cols give denoms.
                    # diag + key-0 strip first: they only need exp, not the
                    # mask, so the PE resumes sooner after the activation.
                    for h in range(2):
                        hsl = slice(65 * h, 65 * h + 65)
                        vsl = slice(65 * h, 65 * h + 65)
                        diag = band[-1]
                        nc.tensor.matmul(
                            pso[:, hsl], probs[h][:, diag[1]:diag[1] + 128],
                            v_sb[:, diag[0], vsl],
                            start=True, stop=False, skip_group_check=True)
                        # key-0 contribution: K=1 outer product
                        nc.tensor.matmul(
                            pso[:, hsl], probs[h][0:1, 384:512],
                            v_sb[0:1, 0, vsl],
                            start=False, stop=False, skip_group_check=True)
                        for j, (kb, off) in enumerate(band[:-1]):
                            nc.tensor.matmul(
                                pso[:, hsl], probs[h][:, off:off + 128],
                                v_sb[:, kb, vsl],
                                start=False, stop=(j == len(band) - 2),
                                skip_group_check=True)
                else:
                    probs = q0_state["probs"]
                    for h in range(2):
                        hsl = slice(65 * h, 65 * h + 65)
                        vsl = slice(65 * h, 65 * h + 65)
                        for j, (kb, off) in enumerate(((1, 0), (0, 128))):
                            nc.tensor.matmul(
                                pso[:, hsl], probs[h][:, off:off + 128],
                                v_sb[:, kb, vsl],
                                start=(j == 0), stop=False,
                                skip_group_check=True)
                        for kb in range(2, NQB):
                            nc.tensor.matmul(
                                pso[0:1, hsl],
                                probs[h][:, 254 + kb:255 + kb],
                                v_sb[:, kb, vsl],
                                start=False, stop=(kb == NQB - 1),
                                skip_group_check=True)

                # normalize + write outn
                recip = tmp.tile([128, 2], f32, tag="recip", name="recip")
                pso_h = pso[:, 0:130].rearrange("p (h c) -> p h c", h=2)
                nc.vector.reciprocal(recip, pso_h[:, :, 64])
                for h in range(2):
                    nc.vector.tensor_scalar_mul(
                        outn_sb[:, qb, 64 * h:64 * h + 64],
                        pso[:, 65 * h:65 * h + 64], recip[:, h:h + 1])

                # transpose -> out-proj -> stage -> DMA
                pstr = psT.tile([128, 128], bf16, tag="tr", name="pstr")
                nc.tensor.transpose(pstr, outn_sb[:, qb, :], id_sb)
                nc.vector.tensor_copy(outt_sb[:, qb, :], pstr)
                stage = tmp.tile([128, HIDDEN], bf16, tag="stage", name="stage")
                for oc in range(2):
                    psp = psO.tile([128, 512], f32, tag="o", name="psp")
                    nc.tensor.matmul(psp, outt_sb[:, qb, :],
                                     wo_sb[:, oc * 512:(oc + 1) * 512],
                                     start=True, stop=True)
                    if oc == 0:
                        nc.vector.tensor_copy(
                            stage[:, oc * 512:(oc + 1) * 512], psp)
                    else:
                        nc.scalar.copy(stage[:, oc * 512:(oc + 1) * 512], psp)
                nc.sync.dma_start(out_d[qb * 128:(qb + 1) * 128, :], stage)

            # ---- projections interleaved with attention ----
            done = 0
            for sc in range(8):
                if 2 <= sc + 1 <= 7:
                    nc.sync.dma_start(xt_sb[:, sc + 1], xt_d[sc + 1])
                ssl = slice(sc * 512, (sc + 1) * 512)

                psq = psS.tile([128, 512], f32, tag="s", name="psq")
                for hc in range(8):
                    nc.tensor.matmul(psq, wq_sb[:, hc, :], xt_sb[:, sc, hc],
                                     start=(hc == 0), stop=(hc == 7))
                nc.vector.tensor_copy(qt_sb[:, ssl], psq)

                psk = psS.tile([128, 512], f32, tag="s", name="psk")
                for hc in range(8):
                    nc.tensor.matmul(psk, wk_sb[:, hc, :], xt_sb[:, sc, hc],
                                     start=(hc == 0), stop=(hc == 7))
                nc.scalar.copy(kt_sb[:, ssl], psk)

                psv = psS.tile([128, 512], f32, tag="s", name="psv")
                for b in range(4):
                    bsl = slice(b * 128, b * 128 + 128)
                    for hc in range(8):
                        nc.tensor.matmul(psv[:, b * 128:b * 128 + 128],
                                         xt_sb[:, sc, hc, bsl], wv_sb[:, hc, :],
                                         start=(hc == 0), stop=(hc == 7),
                                         skip_group_check=True)
                # one strided copy for all 4 blocks x 2 heads
                vdst = v_sb[:, sc * 4:sc * 4 + 4, :].rearrange(
                    "p b (h c) -> p b h c", h=2)
                vsrc = psv.rearrange("p (b h c) -> p b h c", b=4, h=2)
                nc.vector.tensor_copy(vdst[:, :, :, 0:64], vsrc)

                # attention for query blocks whose K/V coverage is complete
                if sc < 7:
                    hi = 4 * sc + 2
                    while done + 1 <= hi:
                        done += 1
                        attention(done)
                else:
                    # qb0's long serial chain starts early and its PE work is
                    # interleaved between the remaining query blocks
                    q0_scores()
                    attention(27)
                    attention(0)
                    for qb in range(28, NQB):
                        attention(qb)

    nc.compile()
    return nc


def kernel(x, Wq, Wk, Wv, Wo):
    from concourse import bass_utils

    x = np.asarray(x)
    B = x.shape[0]
    # xt chunks: [sc, p, c, 512]; hidden h = c*128 + p
    xt = np.ascontiguousarray(
        np.asarray(x)[0].T.astype(BF16).reshape(8, 128, 8, 512)
        .transpose(2, 1, 0, 3))

    def wlayout(W, rs, scale=1.0):
        # W[rs, :].T is [1024 (c p), 128 o] -> [p, c, o]
        wt = (np.asarray(W)[rs, :].T * scale).astype(BF16)
        return np.ascontiguousarray(wt.reshape(8, 128, OC).transpose(1, 0, 2))

    in_maps = []
    for d in range(N_CORES):
        rs = slice(OC * d, OC * (d + 1))
        in_maps.append({
            "xt": xt,
            "wqt": wlayout(Wq, rs, 0.125),
            "wkt": wlayout(Wk, rs),
            "wvt": wlayout(Wv, rs),
            "wot": np.ascontiguousarray(np.asarray(Wo)[:, rs].T.astype(BF16)),
        })

    if "nc" not in _CACHE:
        _CACHE["nc"] = _build()
    nc = _CACHE["nc"]

    res = bass_utils.run_bass_kernel_spmd(
        nc, in_maps, core_ids=list(range(N_CORES)),
        trace=bool(os.environ.get("KERNEL_TRACE")))
    global LAST_RESULTS
    LAST_RESULTS = res

    out = np.zeros((S, HIDDEN), np.float64)
    for r in res.results:
        out += r["partial"].astype(np.float64)
    return out.reshape(B, S, HIDDEN).astype(np.float32)
